# revision 46
# baseline (speedup 1.0000x reference)
"""Trainium2 Bass kernel for paged-KV attention block (QKV proj + RoPE +
paged causal attention + o_proj), tensor-parallel over heads across 8 cores.

Contract: kernel(**inputs) takes the full unsharded inputs (numpy or jax
arrays, keyed as in the reference setup_inputs) and returns the full
[B*Lq, hidden] float32 output.

Sharding (per the tensor-parallel hint):
  - W_pack sharded over heads: each core owns 4 heads of q, k, v rows.
  - KV cache and attention sharded over the same heads.
  - o_proj row-sharded; each core computes a full [T, hidden] partial (in
    fp16) and the partials are summed on the host (replaces the all-reduce
    at zero on-device cost).

Device pipeline (v2 — PE-debottlenecked):
  - q/k computed in transposed [feature, token] layout (wp stationary, hT
    moving); v computed directly in natural [token, feature] layout (hT
    stationary, wp_v moving) so no PE transposes are needed.
  - Softmax denominator: GpSimd (Pool) engine accumulates P tiles
    elementwise; one [1,512] matmul per (head, seq) replaces the per-j-tile
    denominator matmuls (78us of PE time in v1).
  - exp computed per PAIR of kv tiles over a 2-bank PSUM slab — halves the
    Activation-engine per-instruction overhead that paced attention.
  - Causal trim: fresh-kv score tiles only compute q columns >= kv tile
    start; the diagonal block gets an in-place [128,128] triangular mask
    multiply on DVE.
  - o_proj of seq b-1 is interleaved chunk-by-chunk into the QKV phase of
    seq b so each fills the other's pipeline bubbles.
"""

import math
import os

import numpy as np

import concourse.bacc as bacc
import concourse.tile as tile
from concourse import mybir
from concourse.bass_utils import run_bass_kernel_spmd

F32 = mybir.dt.float32
F32R = mybir.dt.float32r
BF16 = mybir.dt.bfloat16
FP16 = mybir.dt.float16

_DT = {"bf16": BF16, "fp16": FP16, "f32r": F32R, "f32": F32}

N_CORES = 8

DT_QKV = _DT[os.environ.get("BASS_KERNEL_DT_QKV", "fp16")]
DT_ATTN = _DT[os.environ.get("BASS_KERNEL_DT_ATTN", "fp16")]
DT_OPROJ = _DT[os.environ.get("BASS_KERNEL_DT_OPROJ", "fp16")]
# den accumulation engine: pool (gpsimd) or vector
DEN_ENG = os.environ.get("BASS_KERNEL_DEN_ENG", "pool")


def build_kernel(B, Lq, H, D, hidden, hist, hpc):
    """Build the SPMD single-core program. hpc = heads per core."""
    assert D == 128 and Lq % 128 == 0 and hist % 256 == 0
    Fqk = hpc * D          # per-core q (or k) feature count = 512
    F3 = 3 * Fqk           # per-core packed qkv features = 1536
    T = B * Lq
    C = hidden
    NCT = C // 128         # contraction tiles = 32
    NJH = hist // 128      # kv tiles in history = 12
    NJF = Lq // 128        # kv tiles fresh = 4
    NJ = NJH + NJF         # 16
    NU = NJ // 2           # pair units = 8
    NOC = hidden // 512    # o_proj column chunks = 8
    NFP = hpc // 2 * 2     # q+k wp pair loads per seq = 4
    LAG = 3                # pair-units between exp and PV consumption
    scale = 1.0 / math.sqrt(D)
    EXP_BIAS = -8.0
    dq, da, do = DT_QKV, DT_ATTN, DT_OPROJ

    nc = bacc.Bacc("TRN2")

    hT = nc.dram_tensor("hT", [C, T], dq, kind="ExternalInput")
    wpT = nc.dram_tensor("wpT", [C, F3], dq, kind="ExternalInput")
    woT = nc.dram_tensor("woT", [Fqk, hidden], do, kind="ExternalInput")
    kTh = nc.dram_tensor("kTh", [hpc, B, D, hist], da, kind="ExternalInput")
    vhd = nc.dram_tensor("vh", [hpc, B, 128, NJH, 128], da, kind="ExternalInput")
    cosT = nc.dram_tensor("cosT", [D, Lq], dq, kind="ExternalInput")
    sinT = nc.dram_tensor("sinT", [D, Lq], dq, kind="ExternalInput")
    Rm16 = nc.dram_tensor("Rm16", [D, D], dq, kind="ExternalInput")
    triM = nc.dram_tensor("triM", [128, 128], da, kind="ExternalInput")
    outp = nc.dram_tensor("outp", [T, hidden], do, kind="ExternalOutput")

    NHC = 4                # hT DMA chunks per seq
    HCT = NCT // NHC       # c-tiles per hT chunk = 8
    with tile.TileContext(nc) as tc:
        with (
            tc.tile_pool(name="const", bufs=1) as p_const,
            tc.tile_pool(name="hTp", bufs=1) as p_hT,
            tc.tile_pool(name="wpp", bufs=2) as p_wp,
            tc.tile_pool(name="qsp", bufs=2) as p_qs,
            tc.tile_pool(name="persist", bufs=1) as p_per,
            tc.tile_pool(name="atp", bufs=2) as p_at,
            tc.tile_pool(name="hist", bufs=1) as p_hist,
            tc.tile_pool(name="Pp", bufs=6) as p_p,
            tc.tile_pool(name="accp", bufs=2) as p_acc,
            tc.tile_pool(name="smalls", bufs=2) as p_small,
            tc.tile_pool(name="wop", bufs=2) as p_wo,
            tc.tile_pool(name="oep", bufs=2) as p_oe,
            tc.tile_pool(name="ps_mm", bufs=2, space="PSUM") as ps_mm,
            tc.tile_pool(name="ps_s", bufs=2, space="PSUM") as ps_s,
            tc.tile_pool(name="ps_pv", bufs=2, space="PSUM") as ps_pv,
        ):
            consts = {}
            den_eng = nc.gpsimd if DEN_ENG == "pool" else nc.vector

            def emit_wpv():
                # v weights in one resident tile, loaded once
                wpv_sb = p_const.tile([128, NCT, Fqk], dq, tag="wpv", name="wpv")
                nc.sync.dma_start(
                    out=wpv_sb,
                    in_=wpT[:, 2 * Fqk: 3 * Fqk].rearrange(
                        "(ct p) f -> p ct f", p=128
                    ),
                )
                consts["wpv"] = wpv_sb

            def emit_consts(skip_wpv=False):
                cos_sb = p_const.tile([D, Lq], dq, tag="cos", name="cos")
                nc.sync.dma_start(out=cos_sb, in_=cosT[:, :])
                sin_sb = p_const.tile([D, Lq], dq, tag="sin", name="sin")
                nc.sync.dma_start(out=sin_sb, in_=sinT[:, :])
                rm_sb = p_const.tile([D, D], dq, tag="rm16", name="rm16")
                nc.sync.dma_start(out=rm_sb, in_=Rm16[:, :])
                tri_sb = p_const.tile([128, 128], da, tag="tri", name="tri")
                nc.sync.dma_start(out=tri_sb, in_=triM[:, :])
                ones_col = p_const.tile([128, 1], da, tag="ones_col",
                                        name="ones_col")
                nc.vector.memset(ones_col, 1.0)
                ones_row16 = p_const.tile([1, 128], da, tag="ones_row16",
                                          name="ones_row16")
                nc.vector.memset(ones_row16, 1.0)
                ebias_sb = p_const.tile([128, 1], F32, tag="ebias", name="ebias")
                nc.vector.memset(ebias_sb, EXP_BIAS)
                consts.update(
                    cos=cos_sb, sin=sin_sb, rm16=rm_sb, tri=tri_sb,
                    ones_col=ones_col, ones_row16=ones_row16, ebias=ebias_sb,
                )
                if not skip_wpv:
                    emit_wpv()

            def load_wp_pair(fp, nsub=1):
                wp_h = []
                nct2 = NCT // 2
                sc = nct2 // nsub
                for wh in range(2):
                    t = p_wp.tile(
                        [128, nct2, 256], dq, tag=f"wp{wh}", name=f"wp{wh}"
                    )
                    for s in range(nsub):
                        nc.sync.dma_start(
                            out=t[:, s * sc: (s + 1) * sc, :],
                            in_=wpT[
                                wh * (C // 2) + s * sc * 128:
                                wh * (C // 2) + (s + 1) * sc * 128,
                                fp * 256: (fp + 1) * 256,
                            ].rearrange("(ct p) f -> p ct f", p=128),
                        )
                    wp_h.append(t)
                return wp_h

            def emit_rope(ft, qs, qrot, krot):
                """RoPE for q/k feature tile ft (0-3 q, 4-7 k)."""
                pr = ps_pv.tile([128, Lq], F32, tag="pv", name="pv")
                nc.tensor.matmul(pr, consts["rm16"], qs, start=True, stop=True)
                tmp1 = p_qs.tile([128, Lq], da, tag="tmp1", name="tmp1")
                nc.vector.tensor_mul(tmp1, qs, consts["cos"])
                tmp = p_qs.tile([128, Lq], da, tag="tmp", name="tmp")
                nc.vector.tensor_mul(tmp, pr, consts["sin"])
                tag = f"qrot{ft}" if ft < hpc else f"krot{ft - hpc}"
                dst = p_per.tile([128, Lq], da, tag=tag)
                nc.vector.tensor_add(dst, tmp1, tmp)
                if ft < hpc:
                    qrot[ft] = dst
                else:
                    krot[ft - hpc] = dst

            def emit_qkv(b, fillers, qrot, krot, vnat, leftover=()):
                """QKV projection for seq b; filler() emits one o_proj chunk
                of the previous seq after each of the 8 stages. `leftover`
                holds the previous attention's tail actions, emitted after
                the first f-pair so they never stall the PE."""
                hT_c = []
                for cc in range(NHC):
                    t = p_hT.tile([128, HCT, Lq], dq, tag=f"hT{cc}",
                                  name=f"hT{cc}")
                    # at cold start, split the first chunk across 4 parallel
                    # DMA queues — a single queue's bandwidth otherwise gates
                    # the first matmul by ~25us
                    nsub = 4 if (b == 0 and cc == 0) else 1
                    sc = HCT // nsub
                    for s in range(nsub):
                        nc.sync.dma_start(
                            out=t[:, s * sc: (s + 1) * sc, :],
                            in_=hT[
                                (cc * HCT + s * sc) * 128:
                                (cc * HCT + (s + 1) * sc) * 128,
                                b * Lq: (b + 1) * Lq,
                            ].rearrange("(ct p) t -> p ct t", p=128),
                        )
                    hT_c.append(t)
                wp0 = load_wp_pair(0, nsub=2 if b == 0 else 1)
                kth_t, vh_t = [], []

                pending = None
                fillers[0]()               # preload first o_proj wo chunk
                for fp in range(NFP):      # q/k feature pairs
                    wp_h = wp0 if fp == 0 else load_wp_pair(fp)
                    if b == 0 and fp == 0:
                        emit_consts(skip_wpv=True)
                    if b == 0 and fp == 2:
                        emit_wpv()
                    if fp == (2 if b == 0 else 1):
                        # prefetch this seq's attention history KV (after the
                        # startup-critical hT/wp DMAs are in flight)
                        for h in range(hpc):
                            kt = p_hist.tile([128, hist], da, tag=f"kth{h}",
                                             name=f"kth{h}")
                            nc.sync.dma_start(out=kt, in_=kTh[h, b])
                            kth_t.append(kt)
                            vt = p_hist.tile([128, NJH, 128], da,
                                             tag=f"vh{h}", name=f"vh{h}")
                            nc.sync.dma_start(out=vt, in_=vhd[h, b])
                            vh_t.append(vt)
                    for sub in range(2):
                        ft = 2 * fp + sub
                        ps = ps_mm.tile([128, Lq], F32, tag="mm", name="mm")
                        for ct in range(NCT):
                            nc.tensor.matmul(
                                ps,
                                wp_h[ct // (NCT // 2)][
                                    :, ct % (NCT // 2),
                                    sub * 128: (sub + 1) * 128,
                                ],
                                hT_c[ct // HCT][:, ct % HCT, :],
                                start=(ct == 0),
                                stop=(ct == NCT - 1),
                            )
                        qs = p_qs.tile([128, Lq], da, tag="qs", name="qs")
                        nc.scalar.copy(qs, ps)
                        if pending is not None:
                            emit_rope(*pending, qrot, krot)
                        pending = (ft, qs)
                    if fp == 0:
                        for fn in leftover:
                            fn()
                    fillers[1 + fp]()      # chunk fp-1 mms + chunk fp preload
                # v phase: natural [token, feature] layout, hT stationary
                for tsub in range(NJF):
                    ps = ps_mm.tile([128, Lq], F32, tag="mm", name="mm")
                    for ct in range(NCT):
                        nc.tensor.matmul(
                            ps[:, 0:Fqk],
                            hT_c[ct // HCT][
                                :, ct % HCT, tsub * 128: (tsub + 1) * 128
                            ],
                            consts["wpv"][:, ct, :],
                            start=(ct == 0),
                            stop=(ct == NCT - 1),
                        )
                    vt = p_per.tile([128, Fqk], da, tag=f"vnat{tsub}")
                    nc.scalar.copy(vt, ps[:, 0:Fqk])
                    vnat[tsub] = vt
                    if pending is not None:
                        emit_rope(*pending, qrot, krot)
                        pending = None
                return kth_t, vh_t

            def emit_attn(b, qrot, krot, vnat, kth_t, vh_t, attnT, fillers):
                """Paged causal attention for seq b (all 4 heads), flattened
                into one deferred-action unit stream so per-head norm tails
                and o_proj filler chunks never block the next head's S/PV
                matmuls on the PE."""
                import heapq
                import itertools
                P_t = {}
                pv_acc = {}
                actions = []
                order = itertools.count()
                gu = 0

                def trim(j):
                    return 0 if j < NJH else (j - NJH) * 128

                def k_lhsT(h, j):
                    if j < NJH:
                        return kth_t[h][:, j * 128: (j + 1) * 128]
                    jj = j - NJH
                    return krot[h][:, jj * 128: (jj + 1) * 128]

                def v_lhsT(h, j):
                    if j < NJH:
                        return vh_t[h][:, j, :]
                    return vnat[j - NJH][:, h * 128: (h + 1) * 128]

                def emit_pv_unit(h, u):
                    def fn():
                        pvh, accD, accP = pv_acc[h]
                        P2 = P_t.pop((h, u))
                        for s in range(2):
                            j = 2 * u + s
                            c = trim(j)
                            nc.tensor.matmul(
                                pvh[:, c:], v_lhsT(h, j), P2[:, s, c:],
                                start=(j == 0), stop=(j == NJ - 1),
                            )
                        # denominator accumulation, split DVE/Pool so neither
                        # engine's serial chain paces the loop
                        c0, c1 = trim(2 * u), trim(2 * u + 1)
                        if u == 0:
                            nc.gpsimd.tensor_add(accP, P2[:, 0, :], P2[:, 1, :])
                        elif u == 1:
                            nc.vector.tensor_add(accD, P2[:, 0, :], P2[:, 1, :])
                        else:
                            nc.vector.tensor_add(
                                accD[:, c0:], accD[:, c0:], P2[:, 0, c0:]
                            )
                            if u % 2 == 1:
                                nc.gpsimd.tensor_add(
                                    accP[:, c1:], accP[:, c1:], P2[:, 1, c1:]
                                )
                            else:
                                nc.vector.tensor_add(
                                    accD[:, c1:], accD[:, c1:], P2[:, 1, c1:]
                                )
                        if u == NU - 1:
                            nc.vector.tensor_add(accD, accD, accP)
                    return fn

                def norm1(h):
                    def fn():
                        _, accD, _ = pv_acc[h]
                        den = ps_mm.tile([128, Lq], F32, tag="mm", name="mm")
                        nc.tensor.matmul(
                            den[0:1, :], consts["ones_col"], accD,
                            start=True, stop=True,
                        )
                        recf = p_small.tile([1, Lq], F32, tag="recf",
                                            name="recf")
                        nc.vector.reciprocal_approx_fast(
                            out=recf, in_=den[0:1, :]
                        )
                        recip = p_small.tile([1, Lq], da, tag="recip",
                                             name="recip")
                        nc.vector.tensor_copy(recip, recf)
                        pv_acc[h] = (pv_acc[h][0], None, recip)
                    return fn

                def norm2(h):
                    def fn():
                        pvh, _, recip = pv_acc.pop(h)
                        bc = ps_mm.tile([128, Lq], F32, tag="mm", name="mm")
                        nc.tensor.matmul(
                            bc, consts["ones_row16"], recip,
                            start=True, stop=True,
                        )
                        bcs = p_small.tile([128, Lq], da, tag="bc", name="bc")
                        nc.vector.tensor_copy(bcs, bc)
                        at = p_at.tile([128, Lq], do, tag=f"attnT{h}",
                                       name=f"attnT{h}")
                        nc.vector.tensor_mul(at, pvh, bcs)
                        attnT[h] = at
                    return fn

                for h in range(hpc):
                    pv_acc[h] = (
                        ps_pv.tile([128, Lq], F32, tag="pv", name="pv"),
                        p_acc.tile([128, Lq], da, tag="accD", name="accD"),
                        p_acc.tile([128, Lq], da, tag="accP", name="accP"),
                    )
                    for u in range(NU):
                        j0, j1 = 2 * u, 2 * u + 1
                        c0, c1 = trim(j0), trim(j1)
                        sp = ps_s.tile([128, 2, Lq], F32, tag="sp", name="sp")
                        nc.tensor.matmul(
                            sp[:, 0, c0:], k_lhsT(h, j0), qrot[h][:, c0:],
                            start=True, stop=True,
                        )
                        nc.tensor.matmul(
                            sp[:, 1, c1:], k_lhsT(h, j1), qrot[h][:, c1:],
                            start=True, stop=True,
                        )
                        P2 = p_p.tile([128, 2, Lq], da, tag="P2", name="P2")
                        nc.scalar.activation(
                            P2[:, :, c0:], sp[:, :, c0:],
                            mybir.ActivationFunctionType.Exp,
                            scale=scale, bias=consts["ebias"][:, :],
                        )
                        for s, j in ((0, j0), (1, j1)):
                            if j >= NJH:
                                c = trim(j)
                                blk = slice(c, c + 128)
                                nc.vector.tensor_mul(
                                    P2[:, s, blk], P2[:, s, blk],
                                    consts["tri"],
                                )
                        P_t[(h, u)] = P2
                        heapq.heappush(
                            actions, (gu + LAG, next(order), emit_pv_unit(h, u))
                        )
                        if u == NU - 1:
                            for off, fn in (
                                (2, norm1(h)), (3, norm2(h)), (4, fillers[h]),
                            ):
                                heapq.heappush(
                                    actions, (gu + LAG + off, next(order), fn)
                                )
                        gu += 1
                        while actions and actions[0][0] <= gu:
                            heapq.heappop(actions)[2]()
                # leftover actions (last head's PV tail + norm) are handed to
                # the caller to interleave behind the next seq's first matmuls
                return [a[2] for a in sorted(actions)]

            def make_oproj_chunk(b, oc, attnT):
                state = {}

                def load():
                    wo_t = p_wo.tile([128, hpc, 512], do, tag="wo", name="wo")
                    nc.sync.dma_start(
                        out=wo_t,
                        in_=woT[:, oc * 512: (oc + 1) * 512].rearrange(
                            "(jt p) o -> p jt o", p=128
                        ),
                    )
                    state["wo"] = wo_t

                def mms():
                    # po accumulators live in the (QKV-phase-idle) ps_s pool.
                    # Both token-subtiles of a half are accumulated into one
                    # 2-bank tile and evicted with a SINGLE copy + DMA, so no
                    # write ever WARs an in-flight eviction of the same tile.
                    wo_t = state["wo"]
                    for half in range(2):
                        spo = ps_s.tile([128, 2, Lq], F32, tag="sp", name="sp")
                        for sub in range(2):
                            tsub = 2 * half + sub
                            for j in range(hpc):
                                nc.tensor.matmul(
                                    spo[:, sub, 0:512],
                                    attnT[j][:, tsub * 128: (tsub + 1) * 128],
                                    wo_t[:, j, :],
                                    start=(j == 0),
                                    stop=(j == hpc - 1),
                                )
                        oe = p_oe.tile([128, 2, 512], do, tag="oe", name="oe")
                        # alternate eviction engine so neither in-order
                        # queue backs up
                        if half == 0:
                            nc.scalar.copy(oe, spo[:, :, 0:512])
                        else:
                            nc.vector.tensor_copy(oe, spo[:, :, 0:512])
                        row = b * Lq + half * 256
                        nc.sync.dma_start(
                            out=outp[
                                row: row + 256, oc * 512: (oc + 1) * 512
                            ].rearrange("(s p) o -> p s o", p=128),
                            in_=oe,
                        )
                return load, mms

            def nop():
                return None

            def make_slots(chunks):
                """Filler slots with one-chunk wo preload lead: slot 0
                preloads chunk 0; slot s runs chunk s-1's matmuls and
                preloads chunk s's wo."""
                def slot(k):
                    def fn():
                        if k < NOC:
                            chunks[k][0]()
                        if k > 0:
                            chunks[k - 1][1]()
                    return fn
                return [slot(k) for k in range(NOC + 1)]

            prev_attnT = None
            leftover = ()
            for b in range(B):
                qrot = [None] * hpc
                krot = [None] * hpc
                vnat = [None] * NJF
                attnT = [None] * hpc
                if prev_attnT is not None:
                    chunks = [make_oproj_chunk(b - 1, oc, prev_attnT)
                              for oc in range(NOC)]
                    slots = make_slots(chunks)
                    # chunks 0-3 fill the q/k stages of qkv(b); chunks 4-7
                    # fill attention(b)'s exp-paced head boundaries
                    qkv_fill = slots[:NFP + 1]
                    attn_fill = slots[NFP + 1:]
                else:
                    qkv_fill = [nop] * (NFP + 1)
                    attn_fill = [nop] * hpc
                kth_t, vh_t = emit_qkv(b, qkv_fill, qrot, krot, vnat,
                                       leftover)
                leftover = emit_attn(b, qrot, krot, vnat, kth_t, vh_t, attnT,
                                     attn_fill)
                prev_attnT = attnT
            chunks = [make_oproj_chunk(B - 1, oc, prev_attnT)
                      for oc in range(NOC)]
            slots = make_slots(chunks)
            slots[0]()
            for fn in leftover:
                fn()
            for fn in slots[1:]:
                fn()

    nc.compile()
    return nc


def _np_dt(d):
    return mybir.dt.np(d)


def prepare_host_inputs(inputs):
    """Shard + relayout the full inputs into 8 per-core input maps."""
    hidden_states = np.ascontiguousarray(
        np.asarray(inputs["hidden_states"], np.float32)
    )
    w_pack = np.asarray(inputs["w_pack"], np.float32)
    w_o = np.asarray(inputs["w_o"], np.float32)
    k_cache = np.asarray(inputs["k_cache"], np.float32)
    v_cache = np.asarray(inputs["v_cache"], np.float32)
    block_offsets = np.asarray(inputs["block_offsets"])
    hist = int(inputs["history_len"])
    Lq = int(inputs["q_len"])
    bs = int(inputs["block_size"])

    B, nblk = block_offsets.shape
    H, D = k_cache.shape[2], k_cache.shape[3]
    hidden = H * D
    T = B * Lq
    assert hidden_states.shape == (T, hidden)
    assert hist % bs == 0 and Lq % bs == 0 and hist % 256 == 0
    hpc = H // N_CORES

    ndq, nda, ndo = _np_dt(DT_QKV), _np_dt(DT_ATTN), _np_dt(DT_OPROJ)

    # shared tensors
    hT = np.ascontiguousarray(hidden_states.T).astype(ndq)

    pos = hist + np.arange(Lq, dtype=np.float64)
    inv_freq = 1.0 / (10000.0 ** (np.arange(0, D, 2, dtype=np.float64) / D))
    ang = pos[None, :] * inv_freq[np.arange(D) % (D // 2), None]  # [D, Lq]
    cosT = np.ascontiguousarray(np.cos(ang)).astype(ndq)
    sinT = np.ascontiguousarray(np.sin(ang)).astype(ndq)

    Rm = np.zeros((D, D), np.float32)
    half = D // 2
    for d in range(half):
        Rm[d + half, d] = -1.0
    for d in range(half, D):
        Rm[d - half, d] = 1.0
    Rm16 = Rm.astype(ndq)

    tri = np.ascontiguousarray(np.triu(np.ones((128, 128)))).astype(nda)

    # paged gather of the history KV (host side = the sharding relayout)
    nhist_blk = hist // bs
    blocks_hist = block_offsets[:, :nhist_blk]
    k_hist = k_cache[blocks_hist].reshape(B, hist, H, D)
    v_hist = v_cache[blocks_hist].reshape(B, hist, H, D)
    NJH = hist // 128

    in_maps = []
    for c in range(N_CORES):
        hs = slice(c * hpc, (c + 1) * hpc)
        rows = np.concatenate(
            [
                q * hidden + np.arange(c * hpc * D, (c + 1) * hpc * D)
                for q in range(3)
            ]
        )
        wpT_c = np.ascontiguousarray(w_pack[rows].T).astype(ndq)
        woT_c = np.ascontiguousarray(
            w_o[:, c * hpc * D: (c + 1) * hpc * D].T
        ).astype(ndo)
        kTh_c = np.ascontiguousarray(
            k_hist[:, :, hs, :].transpose(2, 0, 3, 1)
        ).astype(nda)
        # v history pre-tiled: [h, b, p, j, d] with kv = j*128 + p
        vh_c = np.ascontiguousarray(
            v_hist[:, :, hs, :]
            .reshape(B, NJH, 128, hpc, D)
            .transpose(3, 0, 2, 1, 4)
        ).astype(nda)
        in_maps.append(
            {
                "hT": hT,
                "wpT": wpT_c,
                "woT": woT_c,
                "kTh": kTh_c,
                "vh": vh_c,
                "cosT": cosT,
                "sinT": sinT,
                "Rm16": Rm16,
                "triM": tri,
            }
        )
    meta = dict(B=B, Lq=Lq, H=H, D=D, hidden=hidden, hist=hist, hpc=hpc)
    return in_maps, meta


_NC_CACHE = {}


def run(inputs, trace=False):
    in_maps, meta = prepare_host_inputs(inputs)
    key = tuple(sorted(meta.items()))
    if key not in _NC_CACHE:
        _NC_CACHE[key] = build_kernel(**meta)
    nc = _NC_CACHE[key]
    res = run_bass_kernel_spmd(nc, in_maps, list(range(N_CORES)), trace=trace)
    out = res.results[0]["outp"].astype(np.float64)
    for i in range(1, N_CORES):
        out += res.results[i]["outp"]
    return out.astype(np.float32), res


def kernel(**inputs):
    out, _ = run(inputs, trace=False)
    return out


# revision 47
# speedup vs baseline: 1.1876x; 1.1876x over previous
"""Trainium2 Bass kernel for paged-KV attention block (QKV proj + RoPE +
paged causal attention + o_proj), tensor-parallel over heads across 8 cores.

Contract: kernel(**inputs) takes the full unsharded inputs (numpy or jax
arrays, keyed as in the reference setup_inputs) and returns the full
[B*Lq, hidden] float32 output.

Sharding (per the tensor-parallel hint):
  - W_pack sharded over heads: each core owns 4 heads of q, k, v rows.
  - KV cache and attention sharded over the same heads.
  - o_proj row-sharded; each core computes a full [T, hidden] partial (in
    fp16) and the partials are summed on the host (replaces the all-reduce
    at zero on-device cost).

Device pipeline (v2 — PE-debottlenecked):
  - q/k computed in transposed [feature, token] layout (wp stationary, hT
    moving); v computed directly in natural [token, feature] layout (hT
    stationary, wp_v moving) so no PE transposes are needed.
  - Softmax denominator: GpSimd (Pool) engine accumulates P tiles
    elementwise; one [1,512] matmul per (head, seq) replaces the per-j-tile
    denominator matmuls (78us of PE time in v1).
  - exp computed per PAIR of kv tiles over a 2-bank PSUM slab — halves the
    Activation-engine per-instruction overhead that paced attention.
  - Causal trim: fresh-kv score tiles only compute q columns >= kv tile
    start; the diagonal block gets an in-place [128,128] triangular mask
    multiply on DVE.
  - o_proj of seq b-1 is interleaved chunk-by-chunk into the QKV phase of
    seq b so each fills the other's pipeline bubbles.
"""

import math
import os

import numpy as np

import concourse.bacc as bacc
import concourse.tile as tile
from concourse import mybir
from concourse.bass_utils import run_bass_kernel_spmd

F32 = mybir.dt.float32
F32R = mybir.dt.float32r
BF16 = mybir.dt.bfloat16
FP16 = mybir.dt.float16

_DT = {"bf16": BF16, "fp16": FP16, "f32r": F32R, "f32": F32}

N_CORES = 8

DT_QKV = _DT[os.environ.get("BASS_KERNEL_DT_QKV", "fp16")]
DT_ATTN = _DT[os.environ.get("BASS_KERNEL_DT_ATTN", "fp16")]
DT_OPROJ = _DT[os.environ.get("BASS_KERNEL_DT_OPROJ", "fp16")]
# den accumulation engine: pool (gpsimd) or vector
DEN_ENG = os.environ.get("BASS_KERNEL_DEN_ENG", "pool")


def build_kernel(B, Lq, H, D, hidden, hist, hpc):
    """Build the SPMD single-core program. hpc = heads per core."""
    assert D == 128 and Lq % 128 == 0 and hist % 256 == 0
    Fqk = hpc * D          # per-core q (or k) feature count = 512
    F3 = 3 * Fqk           # per-core packed qkv features = 1536
    T = B * Lq
    C = hidden
    NCT = C // 128         # contraction tiles = 32
    NJH = hist // 128      # kv tiles in history = 12
    NJF = Lq // 128        # kv tiles fresh = 4
    NJ = NJH + NJF         # 16
    NU = NJ // 2           # pair units = 8
    NOC = hidden // 512    # o_proj column chunks = 8
    NFP = hpc // 2 * 2     # q+k wp pair loads per seq = 4
    LAG = 3                # pair-units between exp and PV consumption
    scale = 1.0 / math.sqrt(D)
    EXP_BIAS = -8.0
    dq, da, do = DT_QKV, DT_ATTN, DT_OPROJ

    nc = bacc.Bacc("TRN2")

    hT = nc.dram_tensor("hT", [C, T], dq, kind="ExternalInput")
    wpT = nc.dram_tensor("wpT", [C, F3], dq, kind="ExternalInput")
    woT = nc.dram_tensor("woT", [Fqk, hidden], do, kind="ExternalInput")
    kTh = nc.dram_tensor("kTh", [hpc, B, D, hist], da, kind="ExternalInput")
    vhd = nc.dram_tensor("vh", [hpc, B, 128, NJH, 128], da, kind="ExternalInput")
    cosT = nc.dram_tensor("cosT", [D, Lq], dq, kind="ExternalInput")
    sinT = nc.dram_tensor("sinT", [D, Lq], dq, kind="ExternalInput")
    Rm16 = nc.dram_tensor("Rm16", [D, D], dq, kind="ExternalInput")
    triM = nc.dram_tensor("triM", [128, 128], da, kind="ExternalInput")
    outp = nc.dram_tensor("outp", [T, hidden], do, kind="ExternalOutput")

    NHC = 4                # hT DMA chunks per seq
    HCT = NCT // NHC       # c-tiles per hT chunk = 8
    with tile.TileContext(nc) as tc:
        with (
            tc.tile_pool(name="const", bufs=1) as p_const,
            tc.tile_pool(name="hTp", bufs=1) as p_hT,
            tc.tile_pool(name="wpp", bufs=2) as p_wp,
            tc.tile_pool(name="qsp", bufs=2) as p_qs,
            tc.tile_pool(name="persist", bufs=1) as p_per,
            tc.tile_pool(name="atp", bufs=2) as p_at,
            tc.tile_pool(name="hist", bufs=1) as p_hist,
            tc.tile_pool(name="Pp", bufs=6) as p_p,
            tc.tile_pool(name="accp", bufs=2) as p_acc,
            tc.tile_pool(name="smalls", bufs=2) as p_small,
            tc.tile_pool(name="wop", bufs=2) as p_wo,
            tc.tile_pool(name="oep", bufs=2) as p_oe,
            tc.tile_pool(name="ps_mm", bufs=2, space="PSUM") as ps_mm,
            tc.tile_pool(name="ps_s", bufs=2, space="PSUM") as ps_s,
            tc.tile_pool(name="ps_pv", bufs=2, space="PSUM") as ps_pv,
        ):
            consts = {}
            den_eng = nc.gpsimd if DEN_ENG == "pool" else nc.vector

            def emit_wpv():
                # v weights in one resident tile, loaded once
                wpv_sb = p_const.tile([128, NCT, Fqk], dq, tag="wpv", name="wpv")
                nc.sync.dma_start(
                    out=wpv_sb,
                    in_=wpT[:, 2 * Fqk: 3 * Fqk].rearrange(
                        "(ct p) f -> p ct f", p=128
                    ),
                )
                consts["wpv"] = wpv_sb

            def emit_consts(skip_wpv=False):
                cos_sb = p_const.tile([D, Lq], dq, tag="cos", name="cos")
                nc.sync.dma_start(out=cos_sb, in_=cosT[:, :])
                sin_sb = p_const.tile([D, Lq], dq, tag="sin", name="sin")
                nc.sync.dma_start(out=sin_sb, in_=sinT[:, :])
                rm_sb = p_const.tile([D, D], dq, tag="rm16", name="rm16")
                nc.sync.dma_start(out=rm_sb, in_=Rm16[:, :])
                tri_sb = p_const.tile([128, 128], da, tag="tri", name="tri")
                nc.sync.dma_start(out=tri_sb, in_=triM[:, :])
                ones_col = p_const.tile([128, 1], da, tag="ones_col",
                                        name="ones_col")
                nc.vector.memset(ones_col, 1.0)
                ones_row16 = p_const.tile([1, 128], da, tag="ones_row16",
                                          name="ones_row16")
                nc.vector.memset(ones_row16, 1.0)
                ebias_sb = p_const.tile([128, 1], F32, tag="ebias", name="ebias")
                nc.vector.memset(ebias_sb, EXP_BIAS)
                consts.update(
                    cos=cos_sb, sin=sin_sb, rm16=rm_sb, tri=tri_sb,
                    ones_col=ones_col, ones_row16=ones_row16, ebias=ebias_sb,
                )
                if not skip_wpv:
                    emit_wpv()

            def load_wp_pair(fp):
                wp_h = []
                for wh in range(2):
                    t = p_wp.tile(
                        [128, NCT // 2, 256], dq, tag=f"wp{wh}", name=f"wp{wh}"
                    )
                    nc.sync.dma_start(
                        out=t,
                        in_=wpT[
                            wh * (C // 2): (wh + 1) * (C // 2),
                            fp * 256: (fp + 1) * 256,
                        ].rearrange("(ct p) f -> p ct f", p=128),
                    )
                    wp_h.append(t)
                return wp_h

            def emit_rope(ft, qs, qrot, krot):
                """RoPE for q/k feature tile ft (0-3 q, 4-7 k)."""
                pr = ps_pv.tile([128, Lq], F32, tag="pv", name="pv")
                nc.tensor.matmul(pr, consts["rm16"], qs, start=True, stop=True)
                tmp1 = p_qs.tile([128, Lq], da, tag="tmp1", name="tmp1")
                nc.vector.tensor_mul(tmp1, qs, consts["cos"])
                tmp = p_qs.tile([128, Lq], da, tag="tmp", name="tmp")
                nc.vector.tensor_mul(tmp, pr, consts["sin"])
                tag = f"qrot{ft}" if ft < hpc else f"krot{ft - hpc}"
                dst = p_per.tile([128, Lq], da, tag=tag)
                nc.vector.tensor_add(dst, tmp1, tmp)
                if ft < hpc:
                    qrot[ft] = dst
                else:
                    krot[ft - hpc] = dst

            def emit_qkv(b, fillers, qrot, krot, vnat, leftover=()):
                """QKV projection for seq b; filler() emits one o_proj chunk
                of the previous seq after each of the 8 stages. `leftover`
                holds the previous attention's tail actions, emitted after
                the first f-pair so they never stall the PE."""
                hT_c = []
                for cc in range(NHC):
                    t = p_hT.tile([128, HCT, Lq], dq, tag=f"hT{cc}",
                                  name=f"hT{cc}")
                    nc.sync.dma_start(
                        out=t,
                        in_=hT[
                            cc * HCT * 128: (cc + 1) * HCT * 128,
                            b * Lq: (b + 1) * Lq,
                        ].rearrange("(ct p) t -> p ct t", p=128),
                    )
                    hT_c.append(t)
                wp0 = load_wp_pair(0)
                kth_t, vh_t = [], []

                pending = None
                fillers[0]()               # preload first o_proj wo chunk
                for fp in range(NFP):      # q/k feature pairs
                    wp_h = wp0 if fp == 0 else load_wp_pair(fp)
                    if b == 0 and fp == 0:
                        emit_consts(skip_wpv=True)
                    if b == 0 and fp == 2:
                        emit_wpv()
                    if fp == (2 if b == 0 else 1):
                        # prefetch this seq's attention history KV (after the
                        # startup-critical hT/wp DMAs are in flight)
                        for h in range(hpc):
                            kt = p_hist.tile([128, hist], da, tag=f"kth{h}",
                                             name=f"kth{h}")
                            nc.sync.dma_start(out=kt, in_=kTh[h, b])
                            kth_t.append(kt)
                            vt = p_hist.tile([128, NJH, 128], da,
                                             tag=f"vh{h}", name=f"vh{h}")
                            nc.sync.dma_start(out=vt, in_=vhd[h, b])
                            vh_t.append(vt)
                    for sub in range(2):
                        ft = 2 * fp + sub
                        ps = ps_mm.tile([128, Lq], F32, tag="mm", name="mm")
                        for ct in range(NCT):
                            nc.tensor.matmul(
                                ps,
                                wp_h[ct // (NCT // 2)][
                                    :, ct % (NCT // 2),
                                    sub * 128: (sub + 1) * 128,
                                ],
                                hT_c[ct // HCT][:, ct % HCT, :],
                                start=(ct == 0),
                                stop=(ct == NCT - 1),
                            )
                        qs = p_qs.tile([128, Lq], da, tag="qs", name="qs")
                        nc.scalar.copy(qs, ps)
                        if pending is not None:
                            emit_rope(*pending, qrot, krot)
                        pending = (ft, qs)
                    if fp == 0:
                        for fn in leftover:
                            fn()
                    fillers[1 + fp]()      # chunk fp-1 mms + chunk fp preload
                # v phase: natural [token, feature] layout, hT stationary
                for tsub in range(NJF):
                    ps = ps_mm.tile([128, Lq], F32, tag="mm", name="mm")
                    for ct in range(NCT):
                        nc.tensor.matmul(
                            ps[:, 0:Fqk],
                            hT_c[ct // HCT][
                                :, ct % HCT, tsub * 128: (tsub + 1) * 128
                            ],
                            consts["wpv"][:, ct, :],
                            start=(ct == 0),
                            stop=(ct == NCT - 1),
                        )
                    vt = p_per.tile([128, Fqk], da, tag=f"vnat{tsub}")
                    nc.scalar.copy(vt, ps[:, 0:Fqk])
                    vnat[tsub] = vt
                    if pending is not None:
                        emit_rope(*pending, qrot, krot)
                        pending = None
                return kth_t, vh_t

            def emit_attn(b, qrot, krot, vnat, kth_t, vh_t, attnT, fillers):
                """Paged causal attention for seq b (all 4 heads), flattened
                into one deferred-action unit stream so per-head norm tails
                and o_proj filler chunks never block the next head's S/PV
                matmuls on the PE."""
                import heapq
                import itertools
                P_t = {}
                pv_acc = {}
                actions = []
                order = itertools.count()
                gu = 0

                def trim(j):
                    return 0 if j < NJH else (j - NJH) * 128

                def k_lhsT(h, j):
                    if j < NJH:
                        return kth_t[h][:, j * 128: (j + 1) * 128]
                    jj = j - NJH
                    return krot[h][:, jj * 128: (jj + 1) * 128]

                def v_lhsT(h, j):
                    if j < NJH:
                        return vh_t[h][:, j, :]
                    return vnat[j - NJH][:, h * 128: (h + 1) * 128]

                def emit_pv_unit(h, u):
                    def fn():
                        pvh, accD, accP = pv_acc[h]
                        P2 = P_t.pop((h, u))
                        for s in range(2):
                            j = 2 * u + s
                            c = trim(j)
                            nc.tensor.matmul(
                                pvh[:, c:], v_lhsT(h, j), P2[:, s, c:],
                                start=(j == 0), stop=(j == NJ - 1),
                            )
                        # denominator accumulation, split DVE/Pool so neither
                        # engine's serial chain paces the loop
                        c0, c1 = trim(2 * u), trim(2 * u + 1)
                        if u == 0:
                            nc.gpsimd.tensor_add(accP, P2[:, 0, :], P2[:, 1, :])
                        elif u == 1:
                            nc.vector.tensor_add(accD, P2[:, 0, :], P2[:, 1, :])
                        else:
                            nc.vector.tensor_add(
                                accD[:, c0:], accD[:, c0:], P2[:, 0, c0:]
                            )
                            if u % 2 == 1:
                                nc.gpsimd.tensor_add(
                                    accP[:, c1:], accP[:, c1:], P2[:, 1, c1:]
                                )
                            else:
                                nc.vector.tensor_add(
                                    accD[:, c1:], accD[:, c1:], P2[:, 1, c1:]
                                )
                        if u == NU - 1:
                            nc.vector.tensor_add(accD, accD, accP)
                    return fn

                def norm1(h):
                    def fn():
                        _, accD, _ = pv_acc[h]
                        den = ps_mm.tile([128, Lq], F32, tag="mm", name="mm")
                        nc.tensor.matmul(
                            den[0:1, :], consts["ones_col"], accD,
                            start=True, stop=True,
                        )
                        recf = p_small.tile([1, Lq], F32, tag="recf",
                                            name="recf")
                        nc.vector.reciprocal_approx_fast(
                            out=recf, in_=den[0:1, :]
                        )
                        recip = p_small.tile([1, Lq], da, tag="recip",
                                             name="recip")
                        nc.vector.tensor_copy(recip, recf)
                        pv_acc[h] = (pv_acc[h][0], None, recip)
                    return fn

                def norm2(h):
                    def fn():
                        pvh, _, recip = pv_acc.pop(h)
                        bc = ps_mm.tile([128, Lq], F32, tag="mm", name="mm")
                        nc.tensor.matmul(
                            bc, consts["ones_row16"], recip,
                            start=True, stop=True,
                        )
                        bcs = p_small.tile([128, Lq], da, tag="bc", name="bc")
                        nc.vector.tensor_copy(bcs, bc)
                        at = p_at.tile([128, Lq], do, tag=f"attnT{h}",
                                       name=f"attnT{h}")
                        nc.vector.tensor_mul(at, pvh, bcs)
                        attnT[h] = at
                    return fn

                for h in range(hpc):
                    pv_acc[h] = (
                        ps_pv.tile([128, Lq], F32, tag="pv", name="pv"),
                        p_acc.tile([128, Lq], da, tag="accD", name="accD"),
                        p_acc.tile([128, Lq], da, tag="accP", name="accP"),
                    )
                    for u in range(NU):
                        j0, j1 = 2 * u, 2 * u + 1
                        c0, c1 = trim(j0), trim(j1)
                        sp = ps_s.tile([128, 2, Lq], F32, tag="sp", name="sp")
                        nc.tensor.matmul(
                            sp[:, 0, c0:], k_lhsT(h, j0), qrot[h][:, c0:],
                            start=True, stop=True,
                        )
                        nc.tensor.matmul(
                            sp[:, 1, c1:], k_lhsT(h, j1), qrot[h][:, c1:],
                            start=True, stop=True,
                        )
                        P2 = p_p.tile([128, 2, Lq], da, tag="P2", name="P2")
                        nc.scalar.activation(
                            P2[:, :, c0:], sp[:, :, c0:],
                            mybir.ActivationFunctionType.Exp,
                            scale=scale, bias=consts["ebias"][:, :],
                        )
                        for s, j in ((0, j0), (1, j1)):
                            if j >= NJH:
                                c = trim(j)
                                blk = slice(c, c + 128)
                                nc.vector.tensor_mul(
                                    P2[:, s, blk], P2[:, s, blk],
                                    consts["tri"],
                                )
                        P_t[(h, u)] = P2
                        heapq.heappush(
                            actions, (gu + LAG, next(order), emit_pv_unit(h, u))
                        )
                        if u == NU - 1:
                            for off, fn in (
                                (2, norm1(h)), (3, norm2(h)), (4, fillers[h]),
                            ):
                                heapq.heappush(
                                    actions, (gu + LAG + off, next(order), fn)
                                )
                        gu += 1
                        while actions and actions[0][0] <= gu:
                            heapq.heappop(actions)[2]()
                # leftover actions (last head's PV tail + norm) are handed to
                # the caller to interleave behind the next seq's first matmuls
                return [a[2] for a in sorted(actions)]

            def make_oproj_chunk(b, oc, attnT):
                state = {}

                def load():
                    wo_t = p_wo.tile([128, hpc, 512], do, tag="wo", name="wo")
                    nc.sync.dma_start(
                        out=wo_t,
                        in_=woT[:, oc * 512: (oc + 1) * 512].rearrange(
                            "(jt p) o -> p jt o", p=128
                        ),
                    )
                    state["wo"] = wo_t

                def mms():
                    # po accumulators live in the (QKV-phase-idle) ps_s pool.
                    # Both token-subtiles of a half are accumulated into one
                    # 2-bank tile and evicted with a SINGLE copy + DMA, so no
                    # write ever WARs an in-flight eviction of the same tile.
                    wo_t = state["wo"]
                    for half in range(2):
                        spo = ps_s.tile([128, 2, Lq], F32, tag="sp", name="sp")
                        for sub in range(2):
                            tsub = 2 * half + sub
                            for j in range(hpc):
                                nc.tensor.matmul(
                                    spo[:, sub, 0:512],
                                    attnT[j][:, tsub * 128: (tsub + 1) * 128],
                                    wo_t[:, j, :],
                                    start=(j == 0),
                                    stop=(j == hpc - 1),
                                )
                        oe = p_oe.tile([128, 2, 512], do, tag="oe", name="oe")
                        # alternate eviction engine so neither in-order
                        # queue backs up
                        if half == 0:
                            nc.scalar.copy(oe, spo[:, :, 0:512])
                        else:
                            nc.vector.tensor_copy(oe, spo[:, :, 0:512])
                        row = b * Lq + half * 256
                        nc.sync.dma_start(
                            out=outp[
                                row: row + 256, oc * 512: (oc + 1) * 512
                            ].rearrange("(s p) o -> p s o", p=128),
                            in_=oe,
                        )
                return load, mms

            def nop():
                return None

            def make_slots(chunks):
                """Filler slots with one-chunk wo preload lead: slot 0
                preloads chunk 0; slot s runs chunk s-1's matmuls and
                preloads chunk s's wo."""
                def slot(k):
                    def fn():
                        if k < NOC:
                            chunks[k][0]()
                        if k > 0:
                            chunks[k - 1][1]()
                    return fn
                return [slot(k) for k in range(NOC + 1)]

            prev_attnT = None
            leftover = ()
            for b in range(B):
                qrot = [None] * hpc
                krot = [None] * hpc
                vnat = [None] * NJF
                attnT = [None] * hpc
                if prev_attnT is not None:
                    chunks = [make_oproj_chunk(b - 1, oc, prev_attnT)
                              for oc in range(NOC)]
                    slots = make_slots(chunks)
                    # chunks 0-3 fill the q/k stages of qkv(b); chunks 4-7
                    # fill attention(b)'s exp-paced head boundaries
                    qkv_fill = slots[:NFP + 1]
                    attn_fill = slots[NFP + 1:]
                else:
                    qkv_fill = [nop] * (NFP + 1)
                    attn_fill = [nop] * hpc
                kth_t, vh_t = emit_qkv(b, qkv_fill, qrot, krot, vnat,
                                       leftover)
                leftover = emit_attn(b, qrot, krot, vnat, kth_t, vh_t, attnT,
                                     attn_fill)
                prev_attnT = attnT
            chunks = [make_oproj_chunk(B - 1, oc, prev_attnT)
                      for oc in range(NOC)]
            slots = make_slots(chunks)
            slots[0]()
            for fn in leftover:
                fn()
            for fn in slots[1:]:
                fn()

    nc.compile()
    return nc


def _np_dt(d):
    return mybir.dt.np(d)


def prepare_host_inputs(inputs):
    """Shard + relayout the full inputs into 8 per-core input maps."""
    hidden_states = np.ascontiguousarray(
        np.asarray(inputs["hidden_states"], np.float32)
    )
    w_pack = np.asarray(inputs["w_pack"], np.float32)
    w_o = np.asarray(inputs["w_o"], np.float32)
    k_cache = np.asarray(inputs["k_cache"], np.float32)
    v_cache = np.asarray(inputs["v_cache"], np.float32)
    block_offsets = np.asarray(inputs["block_offsets"])
    hist = int(inputs["history_len"])
    Lq = int(inputs["q_len"])
    bs = int(inputs["block_size"])

    B, nblk = block_offsets.shape
    H, D = k_cache.shape[2], k_cache.shape[3]
    hidden = H * D
    T = B * Lq
    assert hidden_states.shape == (T, hidden)
    assert hist % bs == 0 and Lq % bs == 0 and hist % 256 == 0
    hpc = H // N_CORES

    ndq, nda, ndo = _np_dt(DT_QKV), _np_dt(DT_ATTN), _np_dt(DT_OPROJ)

    # shared tensors
    hT = np.ascontiguousarray(hidden_states.T).astype(ndq)

    pos = hist + np.arange(Lq, dtype=np.float64)
    inv_freq = 1.0 / (10000.0 ** (np.arange(0, D, 2, dtype=np.float64) / D))
    ang = pos[None, :] * inv_freq[np.arange(D) % (D // 2), None]  # [D, Lq]
    cosT = np.ascontiguousarray(np.cos(ang)).astype(ndq)
    sinT = np.ascontiguousarray(np.sin(ang)).astype(ndq)

    Rm = np.zeros((D, D), np.float32)
    half = D // 2
    for d in range(half):
        Rm[d + half, d] = -1.0
    for d in range(half, D):
        Rm[d - half, d] = 1.0
    Rm16 = Rm.astype(ndq)

    tri = np.ascontiguousarray(np.triu(np.ones((128, 128)))).astype(nda)

    # paged gather of the history KV (host side = the sharding relayout)
    nhist_blk = hist // bs
    blocks_hist = block_offsets[:, :nhist_blk]
    k_hist = k_cache[blocks_hist].reshape(B, hist, H, D)
    v_hist = v_cache[blocks_hist].reshape(B, hist, H, D)
    NJH = hist // 128

    in_maps = []
    for c in range(N_CORES):
        hs = slice(c * hpc, (c + 1) * hpc)
        rows = np.concatenate(
            [
                q * hidden + np.arange(c * hpc * D, (c + 1) * hpc * D)
                for q in range(3)
            ]
        )
        wpT_c = np.ascontiguousarray(w_pack[rows].T).astype(ndq)
        woT_c = np.ascontiguousarray(
            w_o[:, c * hpc * D: (c + 1) * hpc * D].T
        ).astype(ndo)
        kTh_c = np.ascontiguousarray(
            k_hist[:, :, hs, :].transpose(2, 0, 3, 1)
        ).astype(nda)
        # v history pre-tiled: [h, b, p, j, d] with kv = j*128 + p
        vh_c = np.ascontiguousarray(
            v_hist[:, :, hs, :]
            .reshape(B, NJH, 128, hpc, D)
            .transpose(3, 0, 2, 1, 4)
        ).astype(nda)
        in_maps.append(
            {
                "hT": hT,
                "wpT": wpT_c,
                "woT": woT_c,
                "kTh": kTh_c,
                "vh": vh_c,
                "cosT": cosT,
                "sinT": sinT,
                "Rm16": Rm16,
                "triM": tri,
            }
        )
    meta = dict(B=B, Lq=Lq, H=H, D=D, hidden=hidden, hist=hist, hpc=hpc)
    return in_maps, meta


_NC_CACHE = {}


def run(inputs, trace=False):
    in_maps, meta = prepare_host_inputs(inputs)
    key = tuple(sorted(meta.items()))
    if key not in _NC_CACHE:
        _NC_CACHE[key] = build_kernel(**meta)
    nc = _NC_CACHE[key]
    res = run_bass_kernel_spmd(nc, in_maps, list(range(N_CORES)), trace=trace)
    out = res.results[0]["outp"].astype(np.float64)
    for i in range(1, N_CORES):
        out += res.results[i]["outp"]
    return out.astype(np.float32), res


def kernel(**inputs):
    out, _ = run(inputs, trace=False)
    return out
